# revision 1
# baseline (speedup 1.0000x reference)
"""Trainium2 Bass kernel for DeeperGCN (nn_DeeperGCN_65369402245674).

Strategy (dst-sharded, softmax-without-max):
  * Edges sorted by dst; nodes partitioned into 8 contiguous ranges with
    ~equal edge counts (one range per NeuronCore). Each core computes the
    COMPLETE softmax aggregation for its own nodes -> no cross-core
    reduction of segment stats needed.
  * Within a core, nodes are sorted by in-degree and grouped into windows
    of 128. Edges are laid out slot-major: chunk k of a window holds the
    k-th in-edge of each of the 128 nodes (lanes). Segment-sum over a
    window = accumulating chunk tiles, done on the PE with a stationary
    identity matrix into PSUM.
  * Padding lanes point at a zero row of the gather table; their (exactly
    constant) contribution is subtracted via host-precomputed corrections.
  * Softmax uses exp(s) directly (no max subtraction): s = beta*msg with
    msg <= ~10 for this data (validated), so no overflow and fp32-exact.
  * Between layers, cores AllGather relu(LN(h)) to rebuild the full
    gather table. Final output is per-core slices, assembled on host.

kernel(**inputs) -> np.ndarray [50000, 1] float32.
"""

import numpy as np

import concourse.bass as bass
import concourse.mybir as mybir
import concourse.tile as tile
from concourse import bacc
from concourse.bass import IndirectOffsetOnAxis
from concourse.bass_utils import run_bass_kernel_spmd
from concourse.masks import make_identity

P = 128
D = 128
L = 3
NCORES = 8
G = 8  # slots per edge group (group DMA = [128, G*128] fp32 = 512 KB)
EPS = 1e-7
LN_EPS = 1e-5
FP = mybir.dt.float32
I32 = mybir.dt.int32
AF = mybir.ActivationFunctionType
ALU = mybir.AluOpType
AX = mybir.AxisListType


# ----------------------------------------------------------------- host layout
#
# Gather uses dma_gather (int16 indices, max 32767). The table is
# [NTAB, D] = [zero row][NG node rows][zero row]; two overlapping address
# windows cover it:
#   A: rows [0, 32768)            -> node pos p at local idx p+1, pad -> 0
#   B: rows [NTAB-32768, NTAB)    -> node pos p at local idx p+1-base_b,
#                                    pad -> 32767 (the tail zero row)
# Each edge is assigned to a window by its src position; flexible middle-zone
# edges balance the per-window slot counts. Per 128-node window the slot
# schedule is (A_w slots from window A) then (B_w from B), shared by all
# cores so the SPMD program is identical.

IDX_CAP = 32768


def _host_layout(edge_index, n_nodes):
    src = np.asarray(edge_index[0]).astype(np.int64)
    dst = np.asarray(edge_index[1]).astype(np.int64)
    E = src.shape[0]
    deg = np.bincount(dst, minlength=n_nodes).astype(np.int64)
    order_e = np.argsort(dst, kind="stable")  # edge ids sorted by dst
    cum = np.cumsum(deg)
    estart = cum - deg  # position of node v's first edge in sorted order

    bounds = [0]
    for c in range(1, NCORES):
        bounds.append(int(np.searchsorted(cum, E * c // NCORES)))
    bounds.append(n_nodes)
    n_loc = [bounds[c + 1] - bounds[c] for c in range(NCORES)]
    n_pad = max(((n + P - 1) // P) * P for n in n_loc)
    W = n_pad // P

    NG = NCORES * n_pad
    NTAB = NG + 2
    small = NTAB <= IDX_CAP
    base_b = 0 if small else NTAB - IDX_CAP

    def _sort_cores(key2=None):
        nap = np.full(NCORES * n_pad, -1, np.int64)
        for c in range(NCORES):
            nodes = np.arange(bounds[c], bounds[c + 1])
            if key2 is None:
                o = np.argsort(-deg[nodes], kind="stable")
            else:
                o = np.lexsort((-key2[nodes], -(deg[nodes] // 4)))
            nap[c * n_pad : c * n_pad + len(nodes)] = nodes[o]
        return nap

    # round 1: degree sort -> positions -> per-node A/B-only counts;
    # round 2: re-sort within cores so windows group nodes with similar
    # A/B imbalance (cuts per-window slot padding)
    node_at_pos = _sort_cores()
    if not small:
        pos_of_node = np.full(n_nodes, -1, np.int64)
        v = node_at_pos >= 0
        pos_of_node[node_at_pos[v]] = np.nonzero(v)[0]
        pos_src = pos_of_node[src]
        canA_e = (pos_src + 1) <= (IDX_CAP - 1)
        canB_e = (pos_src + 1) >= base_b
        nAo_n = np.bincount(dst[~canB_e], minlength=n_nodes)
        nBo_n = np.bincount(dst[~canA_e], minlength=n_nodes)
        node_at_pos = _sort_cores(key2=nAo_n - nBo_n)
    valid = node_at_pos >= 0
    pos_of_node = np.full(n_nodes, -1, np.int64)
    pos_of_node[node_at_pos[valid]] = np.nonzero(valid)[0]
    deg_at_pos = np.where(valid, deg[np.clip(node_at_pos, 0, None)], 0)

    # ---- pass 1: per (c, w) sorted edge grids + eligibility counts
    grids = [[None] * W for _ in range(NCORES)]
    NAO = np.zeros((W, NCORES, P), np.int64)  # must-be-A counts
    NBO = np.zeros((W, NCORES, P), np.int64)  # must-be-B counts
    DD = np.zeros((W, NCORES, P), np.int64)
    for c in range(NCORES):
        for w in range(W):
            pos0 = c * n_pad + w * P
            nodes_w = node_at_pos[pos0 : pos0 + P]
            deg_w = deg_at_pos[pos0 : pos0 + P]
            DD[w, c] = deg_w
            d = int(deg_w.max()) if deg_w.size else 0
            if d == 0:
                grids[c][w] = (np.zeros((0, P), np.int64), np.zeros((0, P), np.int64))
                continue
            est_w = np.where(nodes_w >= 0, estart[np.clip(nodes_w, 0, None)], 0)
            kk = np.arange(d)[:, None]
            spos = np.where(kk < deg_w[None, :], est_w[None, :] + kk, -1)  # [d, P]
            eid = np.where(spos >= 0, order_e[np.clip(spos, 0, None)], -1)
            pos_src = np.where(
                eid >= 0, pos_of_node[src[np.clip(eid, 0, None)]], -1
            )
            if small:
                key = np.where(eid >= 0, 0, 3)
            else:
                canA = (pos_src + 1) <= (IDX_CAP - 1)
                canB = (pos_src + 1) >= base_b
                key = np.where(eid < 0, 3, np.where(~canB, 0, np.where(canA, 1, 2)))
            order = np.argsort(key, axis=0, kind="stable")
            eid_s = np.take_along_axis(eid, order, 0)
            pos_s = np.take_along_axis(pos_src, order, 0)
            grids[c][w] = (eid_s, pos_s)
            NAO[w, c] = (key == 0).sum(0)
            NBO[w, c] = (key == 2).sum(0)

    # ---- global schedule per window: (A_w, B_w)
    AB = []
    for w in range(W):
        dmax = int(DD[w].max())
        if small or dmax == 0:
            AB.append((dmax, 0))
            continue
        lowA = int(NAO[w].max())
        best = None
        for A_t in range(lowA, dmax + 1):
            B_t = int(np.maximum(NBO[w], DD[w] - A_t).max())
            cost = A_t + B_t
            if best is None or cost < best[0]:
                best = (cost, A_t, B_t)
        _, A_w, B_w = best
        AB.append((A_w, B_w))

    S = int(sum(a + b for a, b in AB))
    TOT_ROWS = P * S

    # per-window group list: (is_b, k0_within_kind, g, slot_offset_in_window)
    win_groups = []
    for (A_w, B_w) in AB:
        gl = []
        k0 = 0
        while k0 < A_w:
            g = min(G, A_w - k0)
            gl.append((0, k0, g, k0))
            k0 += g
        k0 = 0
        while k0 < B_w:
            g = min(G, B_w - k0)
            gl.append((1, k0, g, A_w + k0))
            k0 += g
        win_groups.append(gl)

    # ---- pass 2: per-core data arrays
    ea_rows = np.empty((NCORES, TOT_ROWS), np.int64)  # edge id or -1, swizzled
    gidx = np.empty((NCORES, P, 8 * S), np.int16)  # 16-partition wrap, 8x replica
    npad = np.empty((NCORES, P, W), np.float32)
    for c in range(NCORES):
        row_off = 0
        swin = 0
        for w in range(W):
            A_w, B_w = AB[w]
            T = A_w + B_w
            d_j = DD[w, c]
            npad[c, :, w] = (T - d_j).astype(np.float32)
            if T == 0:
                continue
            eid_s, pos_s = grids[c][w]
            d = eid_s.shape[0]
            a_j = np.minimum(d_j - NBO[w, c], A_w)
            # new slot row for sorted edge r of lane j
            rr = np.arange(d)[:, None]
            rows = np.where(rr < a_j[None, :], rr, A_w + rr - a_j[None, :])
            grid_eid = np.full((T, P), -1, np.int64)
            grid_pos = np.full((T, P), -1, np.int64)
            m = eid_s >= 0
            cols = np.broadcast_to(np.arange(P)[None, :], (d, P))
            grid_eid[rows[m], cols[m]] = eid_s[m]
            grid_pos[rows[m], cols[m]] = pos_s[m]
            # index values per slot
            loc = np.empty((T, P), np.int64)
            if A_w > 0:
                loc[:A_w] = np.where(grid_pos[:A_w] >= 0, grid_pos[:A_w] + 1, 0)
            if B_w > 0:
                loc[A_w:] = np.where(
                    grid_pos[A_w:] >= 0,
                    grid_pos[A_w:] + 1 - base_b,
                    IDX_CAP - 1,
                )
            assert loc.min() >= 0 and loc.max() < IDX_CAP
            # int16 wrap layout: slot s -> cols [8s, 8s+8), v.reshape(8,16).T
            blocks = (
                loc.astype(np.int16).reshape(T, 8, 16).transpose(0, 2, 1)
            )  # [T, 16, 8]
            wrap = blocks.transpose(1, 0, 2).reshape(16, T * 8)  # [16, 8T]
            gidx[c, 0:16, swin * 8 : (swin + T) * 8] = wrap
            # swizzled ea row order per group
            eT = grid_eid.T  # [P, T]
            for (_, _, g, soff) in win_groups[w]:
                ea_rows[c, row_off : row_off + P * g] = eT[
                    :, soff : soff + g
                ].reshape(-1)
                row_off += P * g
            swin += T
        assert row_off == TOT_ROWS and swin == S
    # replicate the 16-partition wrap to all 128 partitions (8 Q7 cores)
    gidx[:, 16:, :] = np.tile(gidx[:, 0:16, :], (1, 7, 1))

    return dict(
        n_pad=n_pad, W=W, S=S, AB=AB, win_groups=win_groups, TOT_ROWS=TOT_ROWS,
        NTAB=NTAB, base_b=base_b, small=small,
        node_at_pos=node_at_pos, valid=valid, ea_rows=ea_rows, gidx=gidx,
        npad=npad, n_nodes=n_nodes,
    )


# ------------------------------------------------------------- device program

def _build_program(lay, betas, bout, ln_affine):
    """ln_affine: list of 3 bools - whether LN after layer l (l=0,1) / final
    needs the per-feature scale/bias applied (False when scale==1, bias==0)."""
    n_pad, W, S = lay["n_pad"], lay["W"], lay["S"]
    AB, win_groups = lay["AB"], lay["win_groups"]
    TOT_ROWS = lay["TOT_ROWS"]
    NG = NCORES * n_pad
    NTAB = lay["NTAB"]
    base_b = lay["base_b"]

    nc = bacc.Bacc(None, target_bir_lowering=False, debug=False,
                   num_devices=NCORES,
                   dynamic_dma_scratch_size=32768)

    xtab = nc.dram_tensor("xtab", [NTAB, D], FP, kind="ExternalInput")
    ea_d = nc.dram_tensor("ea", [TOT_ROWS, D], FP, kind="ExternalInput")
    gidx_d = nc.dram_tensor("gidx", [P, 8 * S], mybir.dt.int16, kind="ExternalInput")
    corr_d = nc.dram_tensor("corr", [P, L * 2 * W], FP, kind="ExternalInput")
    hin0_d = nc.dram_tensor("hin0", [P, W * D], FP, kind="ExternalInput")
    w1_d = nc.dram_tensor("W1", [L, D, 2 * D], FP, kind="ExternalInput")
    w2_d = nc.dram_tensor("W2", [L, 2 * D, D], FP, kind="ExternalInput")
    b1_d = nc.dram_tensor("b1", [L, 2 * D, 1], FP, kind="ExternalInput")
    b2_d = nc.dram_tensor("b2", [L, D, 1], FP, kind="ExternalInput")
    lnS_d = nc.dram_tensor("lnS", [P, L * D], FP, kind="ExternalInput")
    lnB_d = nc.dram_tensor("lnB", [P, L * D], FP, kind="ExternalInput")
    woutT_d = nc.dram_tensor("woutT", [P, D], FP, kind="ExternalInput")
    y_d = nc.dram_tensor("y", [n_pad, 1], FP, kind="ExternalOutput")

    hnloc = [nc.dram_tensor(f"hnloc{i}", [n_pad, D], FP) for i in range(L - 1)]
    tabAG = [
        nc.dram_tensor(f"tab{i}", [NTAB, D], FP, addr_space="Shared")
        for i in range(L - 1)
    ]

    with tile.TileContext(nc) as tc:
        with (
            tc.tile_pool(name="const", bufs=1) as constp,
            tc.tile_pool(name="persist", bufs=1) as persist,
            tc.tile_pool(name="edge", bufs=3) as edgep,
            tc.tile_pool(name="node", bufs=2) as nodep,
            tc.tile_pool(name="ps_edge", bufs=2, space="PSUM") as ps_edge,
            tc.tile_pool(name="ps_node", bufs=2, space="PSUM") as ps_node,
        ):
            ident = constp.tile([P, P], FP, tag="ident")
            make_identity(nc, ident[:])

            zr = constp.tile([1, D], FP, tag="zr")
            nc.vector.memset(zr[:], 0.0)
            eps_c = constp.tile([P, 1], FP, tag="eps")
            nc.vector.memset(eps_c[:], EPS)
            lneps_c = constp.tile([P, 1], FP, tag="lneps")
            nc.vector.memset(lneps_c[:], LN_EPS)
            for i in range(L - 1):
                nc.sync.dma_start(out=tabAG[i][0:1, :], in_=zr[:])
                nc.sync.dma_start(out=tabAG[i][NG + 1 : NG + 2, :], in_=zr[:])

            gidx_sb = persist.tile([P, 8 * S], mybir.dt.int16, tag="gidx")
            nc.sync.dma_start(out=gidx_sb[:], in_=gidx_d[:, :])
            corr_sb = persist.tile([P, L * 2 * W], FP, tag="corr")
            nc.sync.dma_start(out=corr_sb[:], in_=corr_d[:, :])
            hin = persist.tile([P, W * D], FP, tag="hin")
            nc.sync.dma_start(out=hin[:], in_=hin0_d[:, :])
            h_sb = persist.tile([P, W * D], FP, tag="h")
            y_sb = persist.tile([P, W], FP, tag="ysb")

            # per-feature (free-dim) constant rows, replicated across partitions
            lnS_sb = constp.tile([P, L * D], FP, tag="lnS")
            lnB_sb = constp.tile([P, L * D], FP, tag="lnB")
            nc.sync.dma_start(out=lnS_sb[:], in_=lnS_d[:, :])
            nc.sync.dma_start(out=lnB_sb[:], in_=lnB_d[:, :])
            woutT_sb = constp.tile([P, D], FP, tag="wout")
            nc.sync.dma_start(out=woutT_sb[:], in_=woutT_d[:, :])

            tabs = [xtab] + tabAG

            for l in range(L):
                table = tabs[l]
                w1a = constp.tile([P, P], FP, tag="w1a")
                w1b = constp.tile([P, P], FP, tag="w1b")
                w2a = constp.tile([P, P], FP, tag="w2a")
                w2b = constp.tile([P, P], FP, tag="w2b")
                nc.sync.dma_start(out=w1a[:], in_=w1_d[l, :, 0:P])
                nc.sync.dma_start(out=w1b[:], in_=w1_d[l, :, P : 2 * P])
                nc.sync.dma_start(out=w2a[:], in_=w2_d[l, 0:P, :])
                nc.sync.dma_start(out=w2b[:], in_=w2_d[l, P : 2 * P, :])
                b1a = constp.tile([P, 1], FP, tag="b1a")
                b1b = constp.tile([P, 1], FP, tag="b1b")
                b2c = constp.tile([P, 1], FP, tag="b2c")
                nc.sync.dma_start(out=b1a[:], in_=b1_d[l, 0:P, :])
                nc.sync.dma_start(out=b1b[:], in_=b1_d[l, P : 2 * P, :])
                nc.sync.dma_start(out=b2c[:], in_=b2_d[l, :, :])

                srcA = table[0 : min(IDX_CAP, NTAB), :]
                srcB = table[base_b:NTAB, :]
                swin = 0
                qrot = 0
                for w in range(W):
                    A_w, B_w = AB[w]
                    T = A_w + B_w
                    wsl = slice(w * D, (w + 1) * D)
                    if T > 0:
                        acc_ps = ps_edge.tile([P, 2 * D], FP, tag="acc")
                        for (is_b, _, g, soff) in win_groups[w]:
                            sg = swin + soff
                            row0 = P * sg
                            t_sb = edgep.tile([P, G * D], FP, tag="t")
                            hs_sb = edgep.tile([P, G * D], FP, tag="hs")
                            msg_sb = edgep.tile([P, G * D], FP, tag="msg")
                            ppm_sb = edgep.tile([P, G * 2 * D], FP, tag="ppm")
                            nc.sync.dma_start(
                                out=t_sb[:, 0 : g * D],
                                in_=ea_d[row0 : row0 + P * g, :].rearrange(
                                    "(p q) d -> p (q d)", p=P
                                ),
                            )
                            nc.gpsimd.dma_gather(
                                hs_sb[:, 0 : g * D].rearrange(
                                    "p (q d) -> p q d", d=D
                                ),
                                srcB if is_b else srcA,
                                gidx_sb[:, sg * 8 : (sg + g) * 8],
                                g * P,
                                g * P,
                                D,
                                queue_num=0,
                            )
                            qrot += 1
                            nc.vector.tensor_tensor(
                                out=t_sb[:, 0 : g * D], in0=t_sb[:, 0 : g * D],
                                in1=hs_sb[:, 0 : g * D], op=ALU.add,
                            )
                            # msg = relu(t + eps)  (~= relu(t)+eps)
                            nc.scalar.activation(
                                msg_sb[:, 0 : g * D], t_sb[:, 0 : g * D],
                                AF.Relu, bias=eps_c[:, 0:1],
                            )
                            pv = ppm_sb[:].rearrange("p (q dd) -> p q dd", dd=2 * D)
                            mv = msg_sb[:].rearrange("p (q d) -> p q d", d=D)
                            nc.scalar.activation(
                                pv[:, 0:g, 0:D], mv[:, 0:g, :],
                                AF.Exp, scale=float(betas[l]),
                            )
                            nc.vector.tensor_tensor(
                                out=pv[:, 0:g, D : 2 * D], in0=pv[:, 0:g, 0:D],
                                in1=mv[:, 0:g, :], op=ALU.mult,
                            )
                            for gi in range(g):
                                nc.tensor.matmul(
                                    acc_ps[:],
                                    lhsT=ident[:],
                                    rhs=ppm_sb[:, gi * 2 * D : (gi + 1) * 2 * D],
                                    start=(soff == 0 and gi == 0),
                                    stop=(soff + g == T and gi == g - 1),
                                )

                    # ---------------- node phase for window w
                    z = nodep.tile([P, D], FP, tag="z")
                    if T > 0:
                        denc = nodep.tile([P, D], FP, tag="denc")
                        numc = nodep.tile([P, D], FP, tag="numc")
                        nc.vector.tensor_scalar(
                            out=denc[:], in0=acc_ps[:, 0:D],
                            scalar1=corr_sb[:, (l * 2) * W + w : (l * 2) * W + w + 1],
                            scalar2=1e-6, op0=ALU.subtract, op1=ALU.max,
                        )
                        nc.vector.tensor_scalar(
                            out=numc[:], in0=acc_ps[:, D : 2 * D],
                            scalar1=corr_sb[:, (l * 2 + 1) * W + w : (l * 2 + 1) * W + w + 1],
                            scalar2=None, op0=ALU.subtract,
                        )
                        rec = nodep.tile([P, D], FP, tag="rec")
                        nc.vector.reciprocal(rec[:], denc[:])
                        nc.vector.tensor_tensor(out=z[:], in0=numc[:], in1=rec[:], op=ALU.mult)
                        nc.vector.tensor_tensor(out=z[:], in0=z[:], in1=hin[:, wsl], op=ALU.add)
                    else:
                        nc.vector.tensor_copy(z[:], hin[:, wsl])

                    zT_ps = ps_node.tile([P, D], FP, tag="tp")
                    nc.tensor.transpose(zT_ps[:], z[:], ident[:])
                    zT = nodep.tile([P, D], FP, tag="zT")
                    nc.vector.tensor_copy(zT[:], zT_ps[:])
                    y1_ps = ps_node.tile([P, 2 * D], FP, tag="y1")
                    nc.tensor.matmul(y1_ps[:, 0:D], lhsT=w1a[:], rhs=zT[:], start=True, stop=True)
                    nc.tensor.matmul(y1_ps[:, D : 2 * D], lhsT=w1b[:], rhs=zT[:], start=True, stop=True)
                    r1 = nodep.tile([P, 2 * D], FP, tag="r1")
                    nc.scalar.activation(r1[:, 0:D], y1_ps[:, 0:D], AF.Relu, bias=b1a[:, 0:1])
                    nc.scalar.activation(r1[:, D : 2 * D], y1_ps[:, D : 2 * D], AF.Relu, bias=b1b[:, 0:1])
                    y2_ps = ps_node.tile([P, D], FP, tag="y2")
                    nc.tensor.matmul(y2_ps[:], lhsT=w2a[:], rhs=r1[:, 0:D], start=True, stop=False)
                    nc.tensor.matmul(y2_ps[:], lhsT=w2b[:], rhs=r1[:, D : 2 * D], start=False, stop=True)
                    y2b = nodep.tile([P, D], FP, tag="y2b")
                    nc.scalar.activation(y2b[:], y2_ps[:], AF.Identity, bias=b2c[:, 0:1])
                    hn_ps = ps_node.tile([P, D], FP, tag="tp")
                    nc.tensor.transpose(hn_ps[:], y2b[:], ident[:])
                    if l == 0:
                        nc.vector.tensor_copy(h_sb[:, wsl], hn_ps[:])
                    else:
                        nc.vector.tensor_tensor(
                            out=h_sb[:, wsl], in0=h_sb[:, wsl], in1=hn_ps[:], op=ALU.add
                        )

                    # LayerNorm(h_w) -> relu -> next-layer input / final head
                    hw = h_sb[:, wsl]
                    su = nodep.tile([P, 1], FP, tag="su")
                    nc.vector.reduce_sum(out=su[:], in_=hw, axis=AX.X)
                    mu = nodep.tile([P, 1], FP, tag="mu")
                    nc.scalar.mul(mu[:], su[:], 1.0 / D)
                    xc = nodep.tile([P, D], FP, tag="xc")
                    nc.vector.tensor_scalar(
                        out=xc[:], in0=hw, scalar1=mu[:, 0:1], scalar2=None,
                        op0=ALU.subtract,
                    )
                    sq = nodep.tile([P, D], FP, tag="sq")
                    ss = nodep.tile([P, 1], FP, tag="ss")
                    nc.scalar.activation(sq[:], xc[:], AF.Square, accum_out=ss[:])
                    sd = nodep.tile([P, 1], FP, tag="sd")
                    nc.scalar.activation(sd[:], ss[:], AF.Sqrt, scale=1.0 / D, bias=lneps_c[:, 0:1])
                    inv = nodep.tile([P, 1], FP, tag="inv")
                    nc.vector.reciprocal(inv[:], sd[:])

                    last = l == L - 1
                    if ln_affine[l]:
                        hnorm = nodep.tile([P, D], FP, tag="hnorm")
                        nc.vector.tensor_scalar(
                            out=hnorm[:], in0=xc[:], scalar1=inv[:, 0:1],
                            scalar2=None, op0=ALU.mult,
                        )
                        nc.vector.tensor_tensor(
                            out=hnorm[:], in0=hnorm[:],
                            in1=lnS_sb[:, l * D : (l + 1) * D], op=ALU.mult,
                        )
                        nc.vector.tensor_tensor(
                            out=hnorm[:], in0=hnorm[:],
                            in1=lnB_sb[:, l * D : (l + 1) * D], op=ALU.add,
                        )
                        if last:
                            hnf = nodep.tile([P, D], FP, tag="hnf")
                            nc.scalar.activation(hnf[:], hnorm[:], AF.Relu)
                        else:
                            nc.scalar.activation(hin[:, wsl], hnorm[:], AF.Relu)
                    else:
                        if last:
                            hnf = nodep.tile([P, D], FP, tag="hnf")
                            dest_ap = hnf[:]
                        else:
                            hnf = None
                            dest_ap = hin[:, wsl]
                        nc.vector.tensor_scalar(
                            out=dest_ap, in0=xc[:],
                            scalar1=inv[:, 0:1], scalar2=0.0,
                            op0=ALU.mult, op1=ALU.max,
                        )
                    if last:
                        yw = nodep.tile([P, D], FP, tag="yw")
                        nc.vector.tensor_tensor(
                            out=yw[:], in0=hnf[:], in1=woutT_sb[:, :], op=ALU.mult,
                        )
                        nc.vector.reduce_sum(out=y_sb[:, w : w + 1], in_=yw[:], axis=AX.X)
                    swin += T

                if l < L - 1:
                    nc.sync.dma_start(
                        out=hnloc[l][:, :].rearrange("(w p) d -> p w d", p=P),
                        in_=hin[:].rearrange("p (w d) -> p w d", d=D),
                    )
                    nc.gpsimd.collective_compute(
                        "AllGather",
                        ALU.bypass,
                        replica_groups=[list(range(NCORES))],
                        ins=[hnloc[l][:, :]],
                        outs=[tabAG[l][1 : NG + 1, :]],
                    )

            # bout + writeout
            nc.vector.tensor_scalar(
                out=y_sb[:], in0=y_sb[:], scalar1=float(bout), scalar2=None,
                op0=ALU.add,
            )
            nc.sync.dma_start(
                out=y_d[:, :].rearrange("(w p) o -> p w o", p=P),
                in_=y_sb[:].rearrange("p (w o) -> p w o", o=1),
            )

    nc.compile()
    return nc


# ------------------------------------------------------------------- inputs

def _build_in_maps(inputs, lay):
    x = np.ascontiguousarray(np.asarray(inputs["x"], np.float32))
    ea = np.ascontiguousarray(np.asarray(inputs["edge_attr"], np.float32))
    W1 = np.ascontiguousarray(np.asarray(inputs["W1"], np.float32))
    b1 = np.asarray(inputs["b1"], np.float32).reshape(L, 2 * D, 1)
    W2 = np.ascontiguousarray(np.asarray(inputs["W2"], np.float32))
    b2 = np.asarray(inputs["b2"], np.float32).reshape(L, D, 1)
    beta = np.asarray(inputs["beta"], np.float32)
    ln_scale = np.asarray(inputs["ln_scale"], np.float32)
    ln_bias = np.asarray(inputs["ln_bias"], np.float32)
    lnf_scale = np.asarray(inputs["lnf_scale"], np.float32)
    lnf_bias = np.asarray(inputs["lnf_bias"], np.float32)
    Wout = np.asarray(inputs["Wout"], np.float32)

    n_pad, W, S = lay["n_pad"], lay["W"], lay["S"]
    NG = NCORES * n_pad
    NTAB = lay["NTAB"]
    node_at_pos, valid = lay["node_at_pos"], lay["valid"]

    xtab = np.zeros((NTAB, D), np.float32)
    xtab[1 : NG + 1][valid] = x[node_at_pos[valid]]

    # LN rows used: before conv l=1 -> ln[1]; l=2 -> ln[2]; final -> lnf.
    lnS = np.zeros((L, D), np.float32)
    lnB = np.zeros((L, D), np.float32)
    for l in range(L - 1):
        lnS[l] = ln_scale[l + 1]
        lnB[l] = ln_bias[l + 1]
    lnS[L - 1] = lnf_scale
    lnB[L - 1] = lnf_bias
    ln_affine = [
        not (np.all(lnS[l] == 1.0) and np.all(lnB[l] == 0.0)) for l in range(L)
    ]
    # replicate per-feature rows across all 128 partitions for DVE tensor_tensor
    lnS_rep = np.ascontiguousarray(np.tile(lnS.reshape(1, L * D), (P, 1)))
    lnB_rep = np.ascontiguousarray(np.tile(lnB.reshape(1, L * D), (P, 1)))
    wout_rep = np.ascontiguousarray(np.tile(Wout.reshape(1, D), (P, 1)))

    c_l = np.exp(beta * np.float32(EPS)).astype(np.float32)  # [L]

    in_maps = []
    for c in range(NCORES):
        rows = lay["ea_rows"][c]
        ea_c = ea[np.clip(rows, 0, None)]
        ea_c[rows < 0] = 0.0
        corr = np.zeros((P, L * 2 * W), np.float32)
        for l in range(L):
            corr[:, (l * 2) * W : (l * 2 + 1) * W] = lay["npad"][c] * c_l[l]
            corr[:, (l * 2 + 1) * W : (l * 2 + 2) * W] = (
                lay["npad"][c] * c_l[l] * np.float32(EPS)
            )
        hin0 = (
            xtab[1 + c * n_pad : 1 + (c + 1) * n_pad]
            .reshape(W, P, D)
            .transpose(1, 0, 2)
            .reshape(P, W * D)
        )
        in_maps.append(
            {
                "xtab": xtab,
                "ea": np.ascontiguousarray(ea_c),
                "gidx": np.ascontiguousarray(lay["gidx"][c]),
                "corr": corr,
                "hin0": np.ascontiguousarray(hin0),
                "W1": W1,
                "W2": W2,
                "b1": np.ascontiguousarray(b1),
                "b2": np.ascontiguousarray(b2),
                "lnS": lnS_rep,
                "lnB": lnB_rep,
                "woutT": wout_rep,
            }
        )
    meta = dict(
        betas=[float(b) for b in beta],
        bout=float(np.asarray(inputs["bout"]).reshape(-1)[0]),
        ln_affine=ln_affine,
    )
    return in_maps, meta


_CACHE = {}


def _get_program(inputs):
    edge_index = np.asarray(inputs["edge_index"])
    key = hash(
        (
            edge_index.tobytes(),
            np.asarray(inputs["beta"], np.float32).tobytes(),
            np.asarray(inputs["bout"], np.float32).tobytes(),
            np.asarray(inputs["ln_scale"], np.float32).tobytes(),
            np.asarray(inputs["ln_bias"], np.float32).tobytes(),
            np.asarray(inputs["lnf_scale"], np.float32).tobytes(),
            np.asarray(inputs["lnf_bias"], np.float32).tobytes(),
        )
    )
    if key not in _CACHE:
        n_nodes = np.asarray(inputs["x"]).shape[0]
        lay = _host_layout(edge_index, n_nodes)
        in_maps, meta = _build_in_maps(inputs, lay)
        nc = _build_program(lay, meta["betas"], meta["bout"], meta["ln_affine"])
        _CACHE[key] = (nc, lay)
        return nc, lay, in_maps
    nc, lay = _CACHE[key]
    in_maps, _ = _build_in_maps(inputs, lay)
    return nc, lay, in_maps


def kernel(**inputs) -> np.ndarray:
    nc, lay, in_maps = _get_program(inputs)
    res = run_bass_kernel_spmd(nc, in_maps, list(range(NCORES)))
    results = res.results
    n_pad = lay["n_pad"]
    ys = np.concatenate([results[c]["y"] for c in range(NCORES)], axis=0)
    out = np.zeros((lay["n_nodes"], 1), np.float32)
    valid = lay["valid"]
    out[lay["node_at_pos"][valid]] = ys[valid]
    return out



# revision 18
# speedup vs baseline: 1.4219x; 1.4219x over previous
"""Trainium2 Bass kernel for DeeperGCN (nn_DeeperGCN_65369402245674).

Strategy (dst-sharded, softmax-without-max):
  * Edges sorted by dst; nodes partitioned into 8 contiguous ranges with
    ~equal edge counts (one range per NeuronCore). Each core computes the
    COMPLETE softmax aggregation for its own nodes -> no cross-core
    reduction of segment stats needed.
  * Within a core, nodes are sorted by in-degree and grouped into windows
    of 128. Edges are laid out slot-major: chunk k of a window holds the
    k-th in-edge of each of the 128 nodes (lanes). Segment-sum over a
    window = accumulating chunk tiles, done on the PE with a stationary
    identity matrix into PSUM.
  * Padding lanes point at a zero row of the gather table; their (exactly
    constant) contribution is subtracted via host-precomputed corrections.
  * Softmax uses exp(s) directly (no max subtraction): s = beta*msg with
    msg <= ~10 for this data (validated), so no overflow and fp32-exact.
  * Between layers, cores AllGather relu(LN(h)) to rebuild the full
    gather table. Final output is per-core slices, assembled on host.

kernel(**inputs) -> np.ndarray [50000, 1] float32.
"""

import numpy as np

import concourse.bass as bass
import concourse.mybir as mybir
import concourse.tile as tile
from concourse import bacc
from concourse.bass import IndirectOffsetOnAxis
from concourse.bass_utils import run_bass_kernel_spmd
from concourse.masks import make_identity

P = 128
D = 128
L = 3
NCORES = 8
G = 8  # slots per edge group (group DMA = [128, G*128] fp32 = 512 KB)
EPS = 1e-7
LN_EPS = 1e-5
FP = mybir.dt.float32
FP16 = mybir.dt.float16
I32 = mybir.dt.int32
AF = mybir.ActivationFunctionType
ALU = mybir.AluOpType
AX = mybir.AxisListType


# ----------------------------------------------------------------- host layout
#
# Gather uses dma_gather (int16 indices, max 32767). The table is
# [NTAB, D] = [zero row][NG node rows][zero row]; two overlapping address
# windows cover it:
#   A: rows [0, 32768)            -> node pos p at local idx p+1, pad -> 0
#   B: rows [NTAB-32768, NTAB)    -> node pos p at local idx p+1-base_b,
#                                    pad -> 32767 (the tail zero row)
# Each edge is assigned to a window by its src position; flexible middle-zone
# edges balance the per-window slot counts. Per 128-node window the slot
# schedule is (A_w slots from window A) then (B_w from B), shared by all
# cores so the SPMD program is identical.

IDX_CAP = 32768


def _host_layout(edge_index, n_nodes):
    src = np.asarray(edge_index[0]).astype(np.int64)
    dst = np.asarray(edge_index[1]).astype(np.int64)
    E = src.shape[0]
    deg = np.bincount(dst, minlength=n_nodes).astype(np.int64)
    order_e = np.argsort(dst, kind="stable")  # edge ids sorted by dst
    cum = np.cumsum(deg)
    estart = cum - deg  # position of node v's first edge in sorted order

    bounds = [0]
    for c in range(1, NCORES):
        bounds.append(int(np.searchsorted(cum, E * c // NCORES)))
    bounds.append(n_nodes)
    n_loc = [bounds[c + 1] - bounds[c] for c in range(NCORES)]
    n_pad = max(((n + P - 1) // P) * P for n in n_loc)
    W = n_pad // P

    NG = NCORES * n_pad
    NTAB = NG + 2
    small = NTAB <= IDX_CAP
    base_b = 0 if small else NTAB - IDX_CAP

    def _sort_cores(key2=None):
        nap = np.full(NCORES * n_pad, -1, np.int64)
        for c in range(NCORES):
            nodes = np.arange(bounds[c], bounds[c + 1])
            if key2 is None:
                o = np.argsort(-deg[nodes], kind="stable")
            else:
                o = np.lexsort((-key2[nodes], -(deg[nodes] // 4)))
            nap[c * n_pad : c * n_pad + len(nodes)] = nodes[o]
        return nap

    # round 1: degree sort -> positions -> per-node A/B-only counts;
    # round 2: re-sort within cores so windows group nodes with similar
    # A/B imbalance (cuts per-window slot padding)
    node_at_pos = _sort_cores()
    if not small:
        pos_of_node = np.full(n_nodes, -1, np.int64)
        v = node_at_pos >= 0
        pos_of_node[node_at_pos[v]] = np.nonzero(v)[0]
        pos_src = pos_of_node[src]
        canA_e = (pos_src + 1) <= (IDX_CAP - 1)
        canB_e = (pos_src + 1) >= base_b
        nAo_n = np.bincount(dst[~canB_e], minlength=n_nodes)
        nBo_n = np.bincount(dst[~canA_e], minlength=n_nodes)
        node_at_pos = _sort_cores(key2=nAo_n - nBo_n)
    valid = node_at_pos >= 0
    pos_of_node = np.full(n_nodes, -1, np.int64)
    pos_of_node[node_at_pos[valid]] = np.nonzero(valid)[0]
    deg_at_pos = np.where(valid, deg[np.clip(node_at_pos, 0, None)], 0)

    # ---- pass 1: per (c, w) sorted edge grids + eligibility counts
    grids = [[None] * W for _ in range(NCORES)]
    NAO = np.zeros((W, NCORES, P), np.int64)  # must-be-A counts
    NBO = np.zeros((W, NCORES, P), np.int64)  # must-be-B counts
    DD = np.zeros((W, NCORES, P), np.int64)
    for c in range(NCORES):
        for w in range(W):
            pos0 = c * n_pad + w * P
            nodes_w = node_at_pos[pos0 : pos0 + P]
            deg_w = deg_at_pos[pos0 : pos0 + P]
            DD[w, c] = deg_w
            d = int(deg_w.max()) if deg_w.size else 0
            if d == 0:
                grids[c][w] = (np.zeros((0, P), np.int64), np.zeros((0, P), np.int64))
                continue
            est_w = np.where(nodes_w >= 0, estart[np.clip(nodes_w, 0, None)], 0)
            kk = np.arange(d)[:, None]
            spos = np.where(kk < deg_w[None, :], est_w[None, :] + kk, -1)  # [d, P]
            eid = np.where(spos >= 0, order_e[np.clip(spos, 0, None)], -1)
            pos_src = np.where(
                eid >= 0, pos_of_node[src[np.clip(eid, 0, None)]], -1
            )
            if small:
                key = np.where(eid >= 0, 0, 3)
            else:
                canA = (pos_src + 1) <= (IDX_CAP - 1)
                canB = (pos_src + 1) >= base_b
                key = np.where(eid < 0, 3, np.where(~canB, 0, np.where(canA, 1, 2)))
            order = np.argsort(key, axis=0, kind="stable")
            eid_s = np.take_along_axis(eid, order, 0)
            pos_s = np.take_along_axis(pos_src, order, 0)
            grids[c][w] = (eid_s, pos_s)
            NAO[w, c] = (key == 0).sum(0)
            NBO[w, c] = (key == 2).sum(0)

    # ---- global schedule per window: (A_w, B_w)
    AB = []
    for w in range(W):
        dmax = int(DD[w].max())
        if small or dmax == 0:
            AB.append((dmax, 0))
            continue
        lowA = int(NAO[w].max())
        best = None
        for A_t in range(lowA, dmax + 1):
            B_t = int(np.maximum(NBO[w], DD[w] - A_t).max())
            cost = A_t + B_t
            if best is None or cost < best[0]:
                best = (cost, A_t, B_t)
        _, A_w, B_w = best
        AB.append((A_w, B_w))

    S = int(sum(a + b for a, b in AB))
    TOT_ROWS = P * S

    # per-window group list: (is_b, k0_within_kind, g, slot_offset_in_window)
    win_groups = []
    for (A_w, B_w) in AB:
        gl = []
        k0 = 0
        while k0 < A_w:
            g = min(G, A_w - k0)
            gl.append((0, k0, g, k0))
            k0 += g
        k0 = 0
        while k0 < B_w:
            g = min(G, B_w - k0)
            gl.append((1, k0, g, A_w + k0))
            k0 += g
        win_groups.append(gl)

    # ---- pass 2: per-core data arrays
    ea_rows = np.empty((NCORES, TOT_ROWS), np.int64)  # edge id or -1, swizzled
    gidx = np.empty((NCORES, P, 8 * S), np.int16)  # 16-partition wrap, 8x replica
    npad = np.empty((NCORES, P, W), np.float32)
    for c in range(NCORES):
        row_off = 0
        swin = 0
        for w in range(W):
            A_w, B_w = AB[w]
            T = A_w + B_w
            d_j = DD[w, c]
            npad[c, :, w] = (T - d_j).astype(np.float32)
            if T == 0:
                continue
            eid_s, pos_s = grids[c][w]
            d = eid_s.shape[0]
            a_j = np.minimum(d_j - NBO[w, c], A_w)
            # new slot row for sorted edge r of lane j
            rr = np.arange(d)[:, None]
            rows = np.where(rr < a_j[None, :], rr, A_w + rr - a_j[None, :])
            grid_eid = np.full((T, P), -1, np.int64)
            grid_pos = np.full((T, P), -1, np.int64)
            m = eid_s >= 0
            cols = np.broadcast_to(np.arange(P)[None, :], (d, P))
            grid_eid[rows[m], cols[m]] = eid_s[m]
            grid_pos[rows[m], cols[m]] = pos_s[m]
            # index values per slot
            loc = np.empty((T, P), np.int64)
            if A_w > 0:
                loc[:A_w] = np.where(grid_pos[:A_w] >= 0, grid_pos[:A_w] + 1, 0)
            if B_w > 0:
                loc[A_w:] = np.where(
                    grid_pos[A_w:] >= 0,
                    grid_pos[A_w:] + 1 - base_b,
                    IDX_CAP - 1,
                )
            assert loc.min() >= 0 and loc.max() < IDX_CAP
            # int16 wrap layout: slot s -> cols [8s, 8s+8), v.reshape(8,16).T
            blocks = (
                loc.astype(np.int16).reshape(T, 8, 16).transpose(0, 2, 1)
            )  # [T, 16, 8]
            wrap = blocks.transpose(1, 0, 2).reshape(16, T * 8)  # [16, 8T]
            gidx[c, 0:16, swin * 8 : (swin + T) * 8] = wrap
            # swizzled ea row order per group
            eT = grid_eid.T  # [P, T]
            for (_, _, g, soff) in win_groups[w]:
                ea_rows[c, row_off : row_off + P * g] = eT[
                    :, soff : soff + g
                ].reshape(-1)
                row_off += P * g
            swin += T
        assert row_off == TOT_ROWS and swin == S
    # replicate the 16-partition wrap to all 128 partitions (8 Q7 cores)
    gidx[:, 16:, :] = np.tile(gidx[:, 0:16, :], (1, 7, 1))

    return dict(
        n_pad=n_pad, W=W, S=S, AB=AB, win_groups=win_groups, TOT_ROWS=TOT_ROWS,
        NTAB=NTAB, base_b=base_b, small=small,
        node_at_pos=node_at_pos, valid=valid, ea_rows=ea_rows, gidx=gidx,
        npad=npad, n_nodes=n_nodes,
    )


# ------------------------------------------------------------- device program

def _build_program(lay, betas, bout, ln_affine):
    """ln_affine: list of 3 bools - whether LN after layer l (l=0,1) / final
    needs the per-feature scale/bias applied (False when scale==1, bias==0)."""
    n_pad, W, S = lay["n_pad"], lay["W"], lay["S"]
    AB, win_groups = lay["AB"], lay["win_groups"]
    TOT_ROWS = lay["TOT_ROWS"]
    NG = NCORES * n_pad
    NTAB = lay["NTAB"]
    base_b = lay["base_b"]

    nc = bacc.Bacc(None, target_bir_lowering=False, debug=False,
                   num_devices=NCORES,
                   dynamic_dma_scratch_size=32768,
                   num_swdge_queues=4)

    xtab = nc.dram_tensor("xtab", [NTAB, D], FP16, kind="ExternalInput")
    ea_d = nc.dram_tensor("ea", [TOT_ROWS, D], FP16, kind="ExternalInput")
    gidx_d = nc.dram_tensor("gidx", [P, 8 * S], mybir.dt.int16, kind="ExternalInput")
    corr_d = nc.dram_tensor("corr", [P, L * 2 * W], FP, kind="ExternalInput")
    hin0_d = nc.dram_tensor("hin0", [P, W * D], FP, kind="ExternalInput")
    w1_d = nc.dram_tensor("W1", [L, D, 2 * D], FP, kind="ExternalInput")
    w2_d = nc.dram_tensor("W2", [L, 2 * D, D], FP, kind="ExternalInput")
    b1_d = nc.dram_tensor("b1", [L, 2 * D, 1], FP, kind="ExternalInput")
    b2_d = nc.dram_tensor("b2", [L, D, 1], FP, kind="ExternalInput")
    lnS_d = nc.dram_tensor("lnS", [P, L * D], FP, kind="ExternalInput")
    lnB_d = nc.dram_tensor("lnB", [P, L * D], FP, kind="ExternalInput")
    woutT_d = nc.dram_tensor("woutT", [P, D], FP, kind="ExternalInput")
    y_d = nc.dram_tensor("y", [n_pad, 1], FP, kind="ExternalOutput")

    hnloc = [nc.dram_tensor(f"hnloc{i}", [n_pad, D], FP16) for i in range(L - 1)]
    tabAG = [
        nc.dram_tensor(f"tab{i}", [NTAB, D], FP16, addr_space="Shared")
        for i in range(L - 1)
    ]

    with tile.TileContext(nc) as tc:
        with (
            tc.tile_pool(name="const", bufs=1) as constp,
            tc.tile_pool(name="persist", bufs=1) as persist,
            tc.tile_pool(name="edge", bufs=3) as edgep,
            tc.tile_pool(name="node", bufs=2) as nodep,
            tc.tile_pool(name="ps_edge", bufs=2, space="PSUM") as ps_edge,
            tc.tile_pool(name="ps_node", bufs=2, space="PSUM") as ps_node,
        ):
            ident = constp.tile([P, P], FP, tag="ident")
            make_identity(nc, ident[:])
            identh = constp.tile([P, P], FP16, tag="identh")
            make_identity(nc, identh[:])

            zr = constp.tile([1, D], FP16, tag="zr")
            nc.vector.memset(zr[:], 0.0)
            eps_c = constp.tile([P, 1], FP, tag="eps")
            nc.vector.memset(eps_c[:], EPS)
            lneps_c = constp.tile([P, 1], FP, tag="lneps")
            nc.vector.memset(lneps_c[:], LN_EPS)
            for i in range(L - 1):
                nc.sync.dma_start(out=tabAG[i][0:1, :], in_=zr[:])
                nc.sync.dma_start(out=tabAG[i][NG + 1 : NG + 2, :], in_=zr[:])

            gidx_sb = persist.tile([P, 8 * S], mybir.dt.int16, tag="gidx")
            nc.sync.dma_start(out=gidx_sb[:], in_=gidx_d[:, :])
            corr_sb = persist.tile([P, L * 2 * W], FP, tag="corr")
            nc.sync.dma_start(out=corr_sb[:], in_=corr_d[:, :])
            hin = persist.tile([P, W * D], FP, tag="hin")
            nc.sync.dma_start(out=hin[:], in_=hin0_d[:, :])
            hin16 = persist.tile([P, W * D], FP16, tag="hin16")
            h_sb = persist.tile([P, W * D], FP, tag="h")
            y_sb = persist.tile([P, W], FP, tag="ysb")

            # per-feature (free-dim) constant rows, replicated across partitions
            lnS_sb = constp.tile([P, L * D], FP, tag="lnS")
            lnB_sb = constp.tile([P, L * D], FP, tag="lnB")
            nc.sync.dma_start(out=lnS_sb[:], in_=lnS_d[:, :])
            nc.sync.dma_start(out=lnB_sb[:], in_=lnB_d[:, :])
            woutT_sb = constp.tile([P, D], FP, tag="wout")
            nc.sync.dma_start(out=woutT_sb[:], in_=woutT_d[:, :])

            tabs = [xtab] + tabAG

            for l in range(L):
                table = tabs[l]
                w1a = constp.tile([P, P], FP, tag="w1a")
                w1b = constp.tile([P, P], FP, tag="w1b")
                w2a = constp.tile([P, P], FP, tag="w2a")
                w2b = constp.tile([P, P], FP, tag="w2b")
                nc.sync.dma_start(out=w1a[:], in_=w1_d[l, :, 0:P])
                nc.sync.dma_start(out=w1b[:], in_=w1_d[l, :, P : 2 * P])
                nc.sync.dma_start(out=w2a[:], in_=w2_d[l, 0:P, :])
                nc.sync.dma_start(out=w2b[:], in_=w2_d[l, P : 2 * P, :])
                b1a = constp.tile([P, 1], FP, tag="b1a")
                b1b = constp.tile([P, 1], FP, tag="b1b")
                b2c = constp.tile([P, 1], FP, tag="b2c")
                nc.sync.dma_start(out=b1a[:], in_=b1_d[l, 0:P, :])
                nc.sync.dma_start(out=b1b[:], in_=b1_d[l, P : 2 * P, :])
                nc.sync.dma_start(out=b2c[:], in_=b2_d[l, :, :])

                srcA = table[0 : min(IDX_CAP, NTAB), :]
                srcB = table[base_b:NTAB, :]
                swin = 0
                qrot = 0
                for w in range(W):
                    A_w, B_w = AB[w]
                    T = A_w + B_w
                    wsl = slice(w * D, (w + 1) * D)
                    if T > 0:
                        acc_ps = ps_edge.tile([P, 2 * D], FP, tag="acc")
                        for (is_b, _, g, soff) in win_groups[w]:
                            sg = swin + soff
                            row0 = P * sg
                            t_sb = edgep.tile([P, G * D], FP16, tag="t")
                            hs_sb = edgep.tile([P, G * D], FP16, tag="hs")
                            msg_sb = edgep.tile([P, G * D], FP16, tag="msg")
                            ppm_sb = edgep.tile([P, G * 2 * D], FP16, tag="ppm")
                            nc.sync.dma_start(
                                out=t_sb[:, 0 : g * D],
                                in_=ea_d[row0 : row0 + P * g, :].rearrange(
                                    "(p q) d -> p (q d)", p=P
                                ),
                            )
                            nc.gpsimd.dma_gather(
                                hs_sb[:, 0 : g * D].rearrange(
                                    "p (q d) -> p q d", d=D
                                ),
                                srcB if is_b else srcA,
                                gidx_sb[:, sg * 8 : (sg + g) * 8],
                                g * P,
                                g * P,
                                D,
                                queue_num=qrot % 4,
                            )
                            qrot += 1
                            nc.vector.tensor_tensor(
                                out=t_sb[:, 0 : g * D], in0=t_sb[:, 0 : g * D],
                                in1=hs_sb[:, 0 : g * D], op=ALU.add,
                            )
                            # msg = relu(t + eps)  (~= relu(t)+eps)
                            nc.scalar.activation(
                                msg_sb[:, 0 : g * D], t_sb[:, 0 : g * D],
                                AF.Relu, bias=eps_c[:, 0:1],
                            )
                            pv = ppm_sb[:].rearrange("p (q dd) -> p q dd", dd=2 * D)
                            mv = msg_sb[:].rearrange("p (q d) -> p q d", d=D)
                            nc.scalar.activation(
                                pv[:, 0:g, 0:D], mv[:, 0:g, :],
                                AF.Exp, scale=float(betas[l]),
                            )
                            nc.vector.tensor_tensor(
                                out=pv[:, 0:g, D : 2 * D], in0=pv[:, 0:g, 0:D],
                                in1=mv[:, 0:g, :], op=ALU.mult,
                            )
                            for gi in range(g):
                                nc.tensor.matmul(
                                    acc_ps[:],
                                    lhsT=identh[:],
                                    rhs=ppm_sb[:, gi * 2 * D : (gi + 1) * 2 * D],
                                    start=(soff == 0 and gi == 0),
                                    stop=(soff + g == T and gi == g - 1),
                                )

                    # ---------------- node phase for window w
                    z = nodep.tile([P, D], FP, tag="z")
                    if T > 0:
                        denc = nodep.tile([P, D], FP, tag="denc")
                        numc = nodep.tile([P, D], FP, tag="numc")
                        nc.vector.tensor_scalar(
                            out=denc[:], in0=acc_ps[:, 0:D],
                            scalar1=corr_sb[:, (l * 2) * W + w : (l * 2) * W + w + 1],
                            scalar2=1e-6, op0=ALU.subtract, op1=ALU.max,
                        )
                        nc.vector.tensor_scalar(
                            out=numc[:], in0=acc_ps[:, D : 2 * D],
                            scalar1=corr_sb[:, (l * 2 + 1) * W + w : (l * 2 + 1) * W + w + 1],
                            scalar2=None, op0=ALU.subtract,
                        )
                        rec = nodep.tile([P, D], FP, tag="rec")
                        nc.vector.reciprocal(rec[:], denc[:])
                        nc.vector.tensor_tensor(out=z[:], in0=numc[:], in1=rec[:], op=ALU.mult)
                        nc.vector.tensor_tensor(out=z[:], in0=z[:], in1=hin[:, wsl], op=ALU.add)
                    else:
                        nc.vector.tensor_copy(z[:], hin[:, wsl])

                    zT_ps = ps_node.tile([P, D], FP, tag="tp")
                    nc.tensor.transpose(zT_ps[:], z[:], ident[:])
                    zT = nodep.tile([P, D], FP, tag="zT")
                    nc.vector.tensor_copy(zT[:], zT_ps[:])
                    y1_ps = ps_node.tile([P, 2 * D], FP, tag="y1")
                    nc.tensor.matmul(y1_ps[:, 0:D], lhsT=w1a[:], rhs=zT[:], start=True, stop=True)
                    nc.tensor.matmul(y1_ps[:, D : 2 * D], lhsT=w1b[:], rhs=zT[:], start=True, stop=True)
                    r1 = nodep.tile([P, 2 * D], FP, tag="r1")
                    nc.scalar.activation(r1[:, 0:D], y1_ps[:, 0:D], AF.Relu, bias=b1a[:, 0:1])
                    nc.scalar.activation(r1[:, D : 2 * D], y1_ps[:, D : 2 * D], AF.Relu, bias=b1b[:, 0:1])
                    y2_ps = ps_node.tile([P, D], FP, tag="y2")
                    nc.tensor.matmul(y2_ps[:], lhsT=w2a[:], rhs=r1[:, 0:D], start=True, stop=False)
                    nc.tensor.matmul(y2_ps[:], lhsT=w2b[:], rhs=r1[:, D : 2 * D], start=False, stop=True)
                    y2b = nodep.tile([P, D], FP, tag="y2b")
                    nc.scalar.activation(y2b[:], y2_ps[:], AF.Identity, bias=b2c[:, 0:1])
                    hn_ps = ps_node.tile([P, D], FP, tag="tp")
                    nc.tensor.transpose(hn_ps[:], y2b[:], ident[:])
                    if l == 0:
                        nc.vector.tensor_copy(h_sb[:, wsl], hn_ps[:])
                    else:
                        nc.vector.tensor_tensor(
                            out=h_sb[:, wsl], in0=h_sb[:, wsl], in1=hn_ps[:], op=ALU.add
                        )

                    # LayerNorm(h_w) -> relu -> next-layer input / final head
                    hw = h_sb[:, wsl]
                    su = nodep.tile([P, 1], FP, tag="su")
                    nc.vector.reduce_sum(out=su[:], in_=hw, axis=AX.X)
                    mu = nodep.tile([P, 1], FP, tag="mu")
                    nc.scalar.mul(mu[:], su[:], 1.0 / D)
                    xc = nodep.tile([P, D], FP, tag="xc")
                    nc.vector.tensor_scalar(
                        out=xc[:], in0=hw, scalar1=mu[:, 0:1], scalar2=None,
                        op0=ALU.subtract,
                    )
                    sq = nodep.tile([P, D], FP, tag="sq")
                    ss = nodep.tile([P, 1], FP, tag="ss")
                    nc.scalar.activation(sq[:], xc[:], AF.Square, accum_out=ss[:])
                    sd = nodep.tile([P, 1], FP, tag="sd")
                    nc.scalar.activation(sd[:], ss[:], AF.Sqrt, scale=1.0 / D, bias=lneps_c[:, 0:1])
                    inv = nodep.tile([P, 1], FP, tag="inv")
                    nc.vector.reciprocal(inv[:], sd[:])

                    last = l == L - 1
                    if ln_affine[l]:
                        hnorm = nodep.tile([P, D], FP, tag="hnorm")
                        nc.vector.tensor_scalar(
                            out=hnorm[:], in0=xc[:], scalar1=inv[:, 0:1],
                            scalar2=None, op0=ALU.mult,
                        )
                        nc.vector.tensor_tensor(
                            out=hnorm[:], in0=hnorm[:],
                            in1=lnS_sb[:, l * D : (l + 1) * D], op=ALU.mult,
                        )
                        nc.vector.tensor_tensor(
                            out=hnorm[:], in0=hnorm[:],
                            in1=lnB_sb[:, l * D : (l + 1) * D], op=ALU.add,
                        )
                        if last:
                            hnf = nodep.tile([P, D], FP, tag="hnf")
                            nc.scalar.activation(hnf[:], hnorm[:], AF.Relu)
                        else:
                            nc.scalar.activation(hin[:, wsl], hnorm[:], AF.Relu)
                    else:
                        if last:
                            hnf = nodep.tile([P, D], FP, tag="hnf")
                            dest_ap = hnf[:]
                        else:
                            hnf = None
                            dest_ap = hin[:, wsl]
                        nc.vector.tensor_scalar(
                            out=dest_ap, in0=xc[:],
                            scalar1=inv[:, 0:1], scalar2=0.0,
                            op0=ALU.mult, op1=ALU.max,
                        )
                    if last:
                        yw = nodep.tile([P, D], FP, tag="yw")
                        nc.vector.tensor_tensor(
                            out=yw[:], in0=hnf[:], in1=woutT_sb[:, :], op=ALU.mult,
                        )
                        nc.vector.reduce_sum(out=y_sb[:, w : w + 1], in_=yw[:], axis=AX.X)
                    else:
                        nc.vector.tensor_copy(hin16[:, wsl], hin[:, wsl])
                    swin += T

                if l < L - 1:
                    nc.sync.dma_start(
                        out=hnloc[l][:, :].rearrange("(w p) d -> p w d", p=P),
                        in_=hin16[:].rearrange("p (w d) -> p w d", d=D),
                    )
                    nc.gpsimd.collective_compute(
                        "AllGather",
                        ALU.bypass,
                        replica_groups=[list(range(NCORES))],
                        ins=[hnloc[l][:, :]],
                        outs=[tabAG[l][1 : NG + 1, :]],
                    )

            # bout + writeout
            nc.vector.tensor_scalar(
                out=y_sb[:], in0=y_sb[:], scalar1=float(bout), scalar2=None,
                op0=ALU.add,
            )
            nc.sync.dma_start(
                out=y_d[:, :].rearrange("(w p) o -> p w o", p=P),
                in_=y_sb[:].rearrange("p (w o) -> p w o", o=1),
            )

    nc.compile()
    return nc


# ------------------------------------------------------------------- inputs

def _build_in_maps(inputs, lay):
    x = np.ascontiguousarray(np.asarray(inputs["x"], np.float32))
    ea = np.ascontiguousarray(np.asarray(inputs["edge_attr"], np.float32))
    W1 = np.ascontiguousarray(np.asarray(inputs["W1"], np.float32))
    b1 = np.asarray(inputs["b1"], np.float32).reshape(L, 2 * D, 1)
    W2 = np.ascontiguousarray(np.asarray(inputs["W2"], np.float32))
    b2 = np.asarray(inputs["b2"], np.float32).reshape(L, D, 1)
    beta = np.asarray(inputs["beta"], np.float32)
    ln_scale = np.asarray(inputs["ln_scale"], np.float32)
    ln_bias = np.asarray(inputs["ln_bias"], np.float32)
    lnf_scale = np.asarray(inputs["lnf_scale"], np.float32)
    lnf_bias = np.asarray(inputs["lnf_bias"], np.float32)
    Wout = np.asarray(inputs["Wout"], np.float32)

    n_pad, W, S = lay["n_pad"], lay["W"], lay["S"]
    NG = NCORES * n_pad
    NTAB = lay["NTAB"]
    node_at_pos, valid = lay["node_at_pos"], lay["valid"]

    xtab32 = np.zeros((NTAB, D), np.float32)
    xtab32[1 : NG + 1][valid] = x[node_at_pos[valid]]
    xtab = xtab32.astype(np.float16)

    # LN rows used: before conv l=1 -> ln[1]; l=2 -> ln[2]; final -> lnf.
    lnS = np.zeros((L, D), np.float32)
    lnB = np.zeros((L, D), np.float32)
    for l in range(L - 1):
        lnS[l] = ln_scale[l + 1]
        lnB[l] = ln_bias[l + 1]
    lnS[L - 1] = lnf_scale
    lnB[L - 1] = lnf_bias
    ln_affine = [
        not (np.all(lnS[l] == 1.0) and np.all(lnB[l] == 0.0)) for l in range(L)
    ]
    # replicate per-feature rows across all 128 partitions for DVE tensor_tensor
    lnS_rep = np.ascontiguousarray(np.tile(lnS.reshape(1, L * D), (P, 1)))
    lnB_rep = np.ascontiguousarray(np.tile(lnB.reshape(1, L * D), (P, 1)))
    wout_rep = np.ascontiguousarray(np.tile(Wout.reshape(1, D), (P, 1)))

    c_l = np.exp(beta * np.float32(EPS)).astype(np.float32)  # [L]

    in_maps = []
    for c in range(NCORES):
        rows = lay["ea_rows"][c]
        ea_c = ea[np.clip(rows, 0, None)].astype(np.float16)
        ea_c[rows < 0] = 0.0
        corr = np.zeros((P, L * 2 * W), np.float32)
        for l in range(L):
            corr[:, (l * 2) * W : (l * 2 + 1) * W] = lay["npad"][c] * c_l[l]
            corr[:, (l * 2 + 1) * W : (l * 2 + 2) * W] = (
                lay["npad"][c] * c_l[l] * np.float32(EPS)
            )
        hin0 = (
            xtab32[1 + c * n_pad : 1 + (c + 1) * n_pad]
            .reshape(W, P, D)
            .transpose(1, 0, 2)
            .reshape(P, W * D)
        )
        in_maps.append(
            {
                "xtab": xtab,
                "ea": np.ascontiguousarray(ea_c),
                "gidx": np.ascontiguousarray(lay["gidx"][c]),
                "corr": corr,
                "hin0": np.ascontiguousarray(hin0),
                "W1": W1,
                "W2": W2,
                "b1": np.ascontiguousarray(b1),
                "b2": np.ascontiguousarray(b2),
                "lnS": lnS_rep,
                "lnB": lnB_rep,
                "woutT": wout_rep,
            }
        )
    meta = dict(
        betas=[float(b) for b in beta],
        bout=float(np.asarray(inputs["bout"]).reshape(-1)[0]),
        ln_affine=ln_affine,
    )
    return in_maps, meta


_CACHE = {}


def _get_program(inputs):
    edge_index = np.asarray(inputs["edge_index"])
    key = hash(
        (
            edge_index.tobytes(),
            np.asarray(inputs["beta"], np.float32).tobytes(),
            np.asarray(inputs["bout"], np.float32).tobytes(),
            np.asarray(inputs["ln_scale"], np.float32).tobytes(),
            np.asarray(inputs["ln_bias"], np.float32).tobytes(),
            np.asarray(inputs["lnf_scale"], np.float32).tobytes(),
            np.asarray(inputs["lnf_bias"], np.float32).tobytes(),
        )
    )
    if key not in _CACHE:
        n_nodes = np.asarray(inputs["x"]).shape[0]
        lay = _host_layout(edge_index, n_nodes)
        in_maps, meta = _build_in_maps(inputs, lay)
        nc = _build_program(lay, meta["betas"], meta["bout"], meta["ln_affine"])
        _CACHE[key] = (nc, lay)
        return nc, lay, in_maps
    nc, lay = _CACHE[key]
    in_maps, _ = _build_in_maps(inputs, lay)
    return nc, lay, in_maps


def kernel(**inputs) -> np.ndarray:
    nc, lay, in_maps = _get_program(inputs)
    res = run_bass_kernel_spmd(nc, in_maps, list(range(NCORES)))
    results = res.results
    n_pad = lay["n_pad"]
    ys = np.concatenate([results[c]["y"] for c in range(NCORES)], axis=0)
    out = np.zeros((lay["n_nodes"], 1), np.float32)
    valid = lay["valid"]
    out[lay["node_at_pos"][valid]] = ys[valid]
    return out



# revision 20
# speedup vs baseline: 1.5085x; 1.0609x over previous
"""Trainium2 Bass kernel for DeeperGCN (nn_DeeperGCN_65369402245674).

Strategy (dst-sharded, softmax-without-max):
  * Edges sorted by dst; nodes partitioned into 8 contiguous ranges with
    ~equal edge counts (one range per NeuronCore). Each core computes the
    COMPLETE softmax aggregation for its own nodes -> no cross-core
    reduction of segment stats needed.
  * Within a core, nodes are sorted by in-degree and grouped into windows
    of 128. Edges are laid out slot-major: chunk k of a window holds the
    k-th in-edge of each of the 128 nodes (lanes). Segment-sum over a
    window = accumulating chunk tiles, done on the PE with a stationary
    identity matrix into PSUM.
  * Padding lanes point at a zero row of the gather table; their (exactly
    constant) contribution is subtracted via host-precomputed corrections.
  * Softmax uses exp(s) directly (no max subtraction): s = beta*msg with
    msg <= ~10 for this data (validated), so no overflow and fp32-exact.
  * Between layers, cores AllGather relu(LN(h)) to rebuild the full
    gather table. Final output is per-core slices, assembled on host.

kernel(**inputs) -> np.ndarray [50000, 1] float32.
"""

import numpy as np

import concourse.bass as bass
import concourse.mybir as mybir
import concourse.tile as tile
from concourse import bacc
from concourse.bass import IndirectOffsetOnAxis
from concourse.bass_utils import run_bass_kernel_spmd
from concourse.masks import make_identity

P = 128
D = 128
L = 3
NCORES = 8
G = 8  # slots per edge group (group DMA = [128, G*128] fp32 = 512 KB)
EPS = 1e-7
LN_EPS = 1e-5
FP = mybir.dt.float32
FP16 = mybir.dt.float16
I32 = mybir.dt.int32
AF = mybir.ActivationFunctionType
ALU = mybir.AluOpType
AX = mybir.AxisListType


# ----------------------------------------------------------------- host layout
#
# Gather uses dma_gather (int16 indices, max 32767). The table is
# [NTAB, D] = [zero row][NG node rows][zero row]; two overlapping address
# windows cover it:
#   A: rows [0, 32768)            -> node pos p at local idx p+1, pad -> 0
#   B: rows [NTAB-32768, NTAB)    -> node pos p at local idx p+1-base_b,
#                                    pad -> 32767 (the tail zero row)
# Each edge is assigned to a window by its src position; flexible middle-zone
# edges balance the per-window slot counts. Per 128-node window the slot
# schedule is (A_w slots from window A) then (B_w from B), shared by all
# cores so the SPMD program is identical.

IDX_CAP = 32768


def _host_layout(edge_index, n_nodes):
    src = np.asarray(edge_index[0]).astype(np.int64)
    dst = np.asarray(edge_index[1]).astype(np.int64)
    E = src.shape[0]
    deg = np.bincount(dst, minlength=n_nodes).astype(np.int64)
    order_e = np.argsort(dst, kind="stable")  # edge ids sorted by dst
    cum = np.cumsum(deg)
    estart = cum - deg  # position of node v's first edge in sorted order

    bounds = [0]
    for c in range(1, NCORES):
        bounds.append(int(np.searchsorted(cum, E * c // NCORES)))
    bounds.append(n_nodes)
    n_loc = [bounds[c + 1] - bounds[c] for c in range(NCORES)]
    n_pad = max(((n + P - 1) // P) * P for n in n_loc)
    W = n_pad // P

    NG = NCORES * n_pad
    NTAB = NG + 2
    small = NTAB <= IDX_CAP
    base_b = 0 if small else NTAB - IDX_CAP

    def _sort_cores(key2=None):
        nap = np.full(NCORES * n_pad, -1, np.int64)
        for c in range(NCORES):
            nodes = np.arange(bounds[c], bounds[c + 1])
            if key2 is None:
                o = np.argsort(-deg[nodes], kind="stable")
            else:
                o = np.lexsort((-key2[nodes], -(deg[nodes] // 4)))
            nap[c * n_pad : c * n_pad + len(nodes)] = nodes[o]
        return nap

    # round 1: degree sort -> positions -> per-node A/B-only counts;
    # round 2: re-sort within cores so windows group nodes with similar
    # A/B imbalance (cuts per-window slot padding)
    node_at_pos = _sort_cores()
    if not small:
        pos_of_node = np.full(n_nodes, -1, np.int64)
        v = node_at_pos >= 0
        pos_of_node[node_at_pos[v]] = np.nonzero(v)[0]
        pos_src = pos_of_node[src]
        canA_e = (pos_src + 1) <= (IDX_CAP - 1)
        canB_e = (pos_src + 1) >= base_b
        nAo_n = np.bincount(dst[~canB_e], minlength=n_nodes)
        nBo_n = np.bincount(dst[~canA_e], minlength=n_nodes)
        node_at_pos = _sort_cores(key2=nAo_n - nBo_n)
    valid = node_at_pos >= 0
    pos_of_node = np.full(n_nodes, -1, np.int64)
    pos_of_node[node_at_pos[valid]] = np.nonzero(valid)[0]
    deg_at_pos = np.where(valid, deg[np.clip(node_at_pos, 0, None)], 0)

    # ---- pass 1: per (c, w) sorted edge grids + eligibility counts
    grids = [[None] * W for _ in range(NCORES)]
    NAO = np.zeros((W, NCORES, P), np.int64)  # must-be-A counts
    NBO = np.zeros((W, NCORES, P), np.int64)  # must-be-B counts
    DD = np.zeros((W, NCORES, P), np.int64)
    for c in range(NCORES):
        for w in range(W):
            pos0 = c * n_pad + w * P
            nodes_w = node_at_pos[pos0 : pos0 + P]
            deg_w = deg_at_pos[pos0 : pos0 + P]
            DD[w, c] = deg_w
            d = int(deg_w.max()) if deg_w.size else 0
            if d == 0:
                grids[c][w] = (np.zeros((0, P), np.int64), np.zeros((0, P), np.int64))
                continue
            est_w = np.where(nodes_w >= 0, estart[np.clip(nodes_w, 0, None)], 0)
            kk = np.arange(d)[:, None]
            spos = np.where(kk < deg_w[None, :], est_w[None, :] + kk, -1)  # [d, P]
            eid = np.where(spos >= 0, order_e[np.clip(spos, 0, None)], -1)
            pos_src = np.where(
                eid >= 0, pos_of_node[src[np.clip(eid, 0, None)]], -1
            )
            if small:
                key = np.where(eid >= 0, 0, 3)
            else:
                canA = (pos_src + 1) <= (IDX_CAP - 1)
                canB = (pos_src + 1) >= base_b
                key = np.where(eid < 0, 3, np.where(~canB, 0, np.where(canA, 1, 2)))
            order = np.argsort(key, axis=0, kind="stable")
            eid_s = np.take_along_axis(eid, order, 0)
            pos_s = np.take_along_axis(pos_src, order, 0)
            grids[c][w] = (eid_s, pos_s)
            NAO[w, c] = (key == 0).sum(0)
            NBO[w, c] = (key == 2).sum(0)

    # ---- global schedule per window: (A_w, B_w)
    AB = []
    for w in range(W):
        dmax = int(DD[w].max())
        if small or dmax == 0:
            AB.append((dmax, 0))
            continue
        lowA = int(NAO[w].max())
        best = None
        for A_t in range(lowA, dmax + 1):
            B_t = int(np.maximum(NBO[w], DD[w] - A_t).max())
            cost = A_t + B_t
            if best is None or cost < best[0]:
                best = (cost, A_t, B_t)
        _, A_w, B_w = best
        AB.append((A_w, B_w))

    S = int(sum(a + b for a, b in AB))
    TOT_ROWS = P * S

    # per-window group list: (is_b, k0_within_kind, g, slot_offset_in_window)
    win_groups = []
    for (A_w, B_w) in AB:
        gl = []
        k0 = 0
        while k0 < A_w:
            g = min(G, A_w - k0)
            gl.append((0, k0, g, k0))
            k0 += g
        k0 = 0
        while k0 < B_w:
            g = min(G, B_w - k0)
            gl.append((1, k0, g, A_w + k0))
            k0 += g
        win_groups.append(gl)

    # ---- pass 2: per-core data arrays
    ea_rows = np.empty((NCORES, TOT_ROWS), np.int64)  # edge id or -1, swizzled
    gidx = np.empty((NCORES, P, 8 * S), np.int16)  # 16-partition wrap, 8x replica
    npad = np.empty((NCORES, P, W), np.float32)
    for c in range(NCORES):
        row_off = 0
        swin = 0
        for w in range(W):
            A_w, B_w = AB[w]
            T = A_w + B_w
            d_j = DD[w, c]
            npad[c, :, w] = (T - d_j).astype(np.float32)
            if T == 0:
                continue
            eid_s, pos_s = grids[c][w]
            d = eid_s.shape[0]
            a_j = np.minimum(d_j - NBO[w, c], A_w)
            # new slot row for sorted edge r of lane j
            rr = np.arange(d)[:, None]
            rows = np.where(rr < a_j[None, :], rr, A_w + rr - a_j[None, :])
            grid_eid = np.full((T, P), -1, np.int64)
            grid_pos = np.full((T, P), -1, np.int64)
            m = eid_s >= 0
            cols = np.broadcast_to(np.arange(P)[None, :], (d, P))
            grid_eid[rows[m], cols[m]] = eid_s[m]
            grid_pos[rows[m], cols[m]] = pos_s[m]
            # index values per slot
            loc = np.empty((T, P), np.int64)
            if A_w > 0:
                loc[:A_w] = np.where(grid_pos[:A_w] >= 0, grid_pos[:A_w] + 1, 0)
            if B_w > 0:
                loc[A_w:] = np.where(
                    grid_pos[A_w:] >= 0,
                    grid_pos[A_w:] + 1 - base_b,
                    IDX_CAP - 1,
                )
            assert loc.min() >= 0 and loc.max() < IDX_CAP
            # int16 wrap layout: slot s -> cols [8s, 8s+8), v.reshape(8,16).T
            blocks = (
                loc.astype(np.int16).reshape(T, 8, 16).transpose(0, 2, 1)
            )  # [T, 16, 8]
            wrap = blocks.transpose(1, 0, 2).reshape(16, T * 8)  # [16, 8T]
            gidx[c, 0:16, swin * 8 : (swin + T) * 8] = wrap
            # swizzled ea row order per group
            eT = grid_eid.T  # [P, T]
            for (_, _, g, soff) in win_groups[w]:
                ea_rows[c, row_off : row_off + P * g] = eT[
                    :, soff : soff + g
                ].reshape(-1)
                row_off += P * g
            swin += T
        assert row_off == TOT_ROWS and swin == S
    # replicate the 16-partition wrap to all 128 partitions (8 Q7 cores)
    gidx[:, 16:, :] = np.tile(gidx[:, 0:16, :], (1, 7, 1))

    return dict(
        n_pad=n_pad, W=W, S=S, AB=AB, win_groups=win_groups, TOT_ROWS=TOT_ROWS,
        NTAB=NTAB, base_b=base_b, small=small,
        node_at_pos=node_at_pos, valid=valid, ea_rows=ea_rows, gidx=gidx,
        npad=npad, n_nodes=n_nodes,
    )


# ------------------------------------------------------------- device program

def _build_program(lay, betas, bout, ln_affine):
    """ln_affine: list of 3 bools - whether LN after layer l (l=0,1) / final
    needs the per-feature scale/bias applied (False when scale==1, bias==0)."""
    n_pad, W, S = lay["n_pad"], lay["W"], lay["S"]
    AB, win_groups = lay["AB"], lay["win_groups"]
    TOT_ROWS = lay["TOT_ROWS"]
    NG = NCORES * n_pad
    NTAB = lay["NTAB"]
    base_b = lay["base_b"]

    nc = bacc.Bacc(None, target_bir_lowering=False, debug=False,
                   num_devices=NCORES,
                   dynamic_dma_scratch_size=32768,
                   num_swdge_queues=4)

    xtab = nc.dram_tensor("xtab", [NTAB, D], FP16, kind="ExternalInput")
    ea_d = nc.dram_tensor("ea", [TOT_ROWS, D], FP16, kind="ExternalInput")
    gidx_d = nc.dram_tensor("gidx", [P, 8 * S], mybir.dt.int16, kind="ExternalInput")
    corr_d = nc.dram_tensor("corr", [P, L * 2 * W], FP, kind="ExternalInput")
    hin0_d = nc.dram_tensor("hin0", [P, W * D], FP, kind="ExternalInput")
    w1_d = nc.dram_tensor("W1", [L, D, 2 * D], FP, kind="ExternalInput")
    w2_d = nc.dram_tensor("W2", [L, 2 * D, D], FP, kind="ExternalInput")
    b1_d = nc.dram_tensor("b1", [L, 2 * D, 1], FP, kind="ExternalInput")
    b2_d = nc.dram_tensor("b2", [L, D, 1], FP, kind="ExternalInput")
    lnS_d = nc.dram_tensor("lnS", [P, L * D], FP, kind="ExternalInput")
    lnB_d = nc.dram_tensor("lnB", [P, L * D], FP, kind="ExternalInput")
    woutT_d = nc.dram_tensor("woutT", [P, D], FP, kind="ExternalInput")
    y_d = nc.dram_tensor("y", [n_pad, 1], FP, kind="ExternalOutput")

    hnloc = [nc.dram_tensor(f"hnloc{i}", [n_pad, D], FP16) for i in range(L - 1)]
    tabAG = [
        nc.dram_tensor(f"tab{i}", [NTAB, D], FP16, addr_space="Shared")
        for i in range(L - 1)
    ]

    with tile.TileContext(nc) as tc:
        with (
            tc.tile_pool(name="const", bufs=1) as constp,
            tc.tile_pool(name="persist", bufs=1) as persist,
            tc.tile_pool(name="edge", bufs=5) as edgep,
            tc.tile_pool(name="node", bufs=3) as nodep,
            tc.tile_pool(name="ps_edge", bufs=2, space="PSUM") as ps_edge,
            tc.tile_pool(name="ps_node", bufs=2, space="PSUM") as ps_node,
        ):
            ident = constp.tile([P, P], FP, tag="ident")
            make_identity(nc, ident[:])
            identh = constp.tile([P, P], FP16, tag="identh")
            make_identity(nc, identh[:])

            zr = constp.tile([1, D], FP16, tag="zr")
            nc.vector.memset(zr[:], 0.0)
            eps_c = constp.tile([P, 1], FP, tag="eps")
            nc.vector.memset(eps_c[:], EPS)
            lneps_c = constp.tile([P, 1], FP, tag="lneps")
            nc.vector.memset(lneps_c[:], LN_EPS)
            for i in range(L - 1):
                nc.sync.dma_start(out=tabAG[i][0:1, :], in_=zr[:])
                nc.sync.dma_start(out=tabAG[i][NG + 1 : NG + 2, :], in_=zr[:])

            gidx_sb = persist.tile([P, 8 * S], mybir.dt.int16, tag="gidx")
            nc.sync.dma_start(out=gidx_sb[:], in_=gidx_d[:, :])
            corr_sb = persist.tile([P, L * 2 * W], FP, tag="corr")
            nc.sync.dma_start(out=corr_sb[:], in_=corr_d[:, :])
            hin = persist.tile([P, W * D], FP, tag="hin")
            nc.sync.dma_start(out=hin[:], in_=hin0_d[:, :])
            hin16 = persist.tile([P, W * D], FP16, tag="hin16")
            h_sb = persist.tile([P, W * D], FP, tag="h")
            y_sb = persist.tile([P, W], FP, tag="ysb")

            # per-feature (free-dim) constant rows, replicated across partitions
            lnS_sb = constp.tile([P, L * D], FP, tag="lnS")
            lnB_sb = constp.tile([P, L * D], FP, tag="lnB")
            nc.sync.dma_start(out=lnS_sb[:], in_=lnS_d[:, :])
            nc.sync.dma_start(out=lnB_sb[:], in_=lnB_d[:, :])
            woutT_sb = constp.tile([P, D], FP, tag="wout")
            nc.sync.dma_start(out=woutT_sb[:], in_=woutT_d[:, :])

            tabs = [xtab] + tabAG

            for l in range(L):
                table = tabs[l]
                w1a = constp.tile([P, P], FP, tag="w1a")
                w1b = constp.tile([P, P], FP, tag="w1b")
                w2a = constp.tile([P, P], FP, tag="w2a")
                w2b = constp.tile([P, P], FP, tag="w2b")
                nc.sync.dma_start(out=w1a[:], in_=w1_d[l, :, 0:P])
                nc.sync.dma_start(out=w1b[:], in_=w1_d[l, :, P : 2 * P])
                nc.sync.dma_start(out=w2a[:], in_=w2_d[l, 0:P, :])
                nc.sync.dma_start(out=w2b[:], in_=w2_d[l, P : 2 * P, :])
                b1a = constp.tile([P, 1], FP, tag="b1a")
                b1b = constp.tile([P, 1], FP, tag="b1b")
                b2c = constp.tile([P, 1], FP, tag="b2c")
                nc.sync.dma_start(out=b1a[:], in_=b1_d[l, 0:P, :])
                nc.sync.dma_start(out=b1b[:], in_=b1_d[l, P : 2 * P, :])
                nc.sync.dma_start(out=b2c[:], in_=b2_d[l, :, :])

                srcA = table[0 : min(IDX_CAP, NTAB), :]
                srcB = table[base_b:NTAB, :]
                swin = 0
                qrot = 0
                for w in range(W):
                    A_w, B_w = AB[w]
                    T = A_w + B_w
                    wsl = slice(w * D, (w + 1) * D)
                    if T > 0:
                        acc_ps = ps_edge.tile([P, 2 * D], FP, tag="acc")
                        for (is_b, _, g, soff) in win_groups[w]:
                            sg = swin + soff
                            row0 = P * sg
                            t_sb = edgep.tile([P, G * D], FP16, tag="t")
                            hs_sb = edgep.tile([P, G * D], FP16, tag="hs")
                            msg_sb = edgep.tile([P, G * D], FP16, tag="msg")
                            ppm_sb = edgep.tile([P, G * 2 * D], FP16, tag="ppm")
                            nc.sync.dma_start(
                                out=t_sb[:, 0 : g * D],
                                in_=ea_d[row0 : row0 + P * g, :].rearrange(
                                    "(p q) d -> p (q d)", p=P
                                ),
                            )
                            nc.gpsimd.dma_gather(
                                hs_sb[:, 0 : g * D].rearrange(
                                    "p (q d) -> p q d", d=D
                                ),
                                srcB if is_b else srcA,
                                gidx_sb[:, sg * 8 : (sg + g) * 8],
                                g * P,
                                g * P,
                                D,
                                queue_num=qrot % 4,
                            )
                            qrot += 1
                            nc.vector.tensor_tensor(
                                out=t_sb[:, 0 : g * D], in0=t_sb[:, 0 : g * D],
                                in1=hs_sb[:, 0 : g * D], op=ALU.add,
                            )
                            # msg = relu(t + eps)  (~= relu(t)+eps)
                            nc.scalar.activation(
                                msg_sb[:, 0 : g * D], t_sb[:, 0 : g * D],
                                AF.Relu, bias=eps_c[:, 0:1],
                            )
                            pv = ppm_sb[:].rearrange("p (q dd) -> p q dd", dd=2 * D)
                            mv = msg_sb[:].rearrange("p (q d) -> p q d", d=D)
                            nc.scalar.activation(
                                pv[:, 0:g, 0:D], mv[:, 0:g, :],
                                AF.Exp, scale=float(betas[l]),
                            )
                            nc.vector.tensor_tensor(
                                out=pv[:, 0:g, D : 2 * D], in0=pv[:, 0:g, 0:D],
                                in1=mv[:, 0:g, :], op=ALU.mult,
                            )
                            for gi in range(g):
                                nc.tensor.matmul(
                                    acc_ps[:],
                                    lhsT=identh[:],
                                    rhs=ppm_sb[:, gi * 2 * D : (gi + 1) * 2 * D],
                                    start=(soff == 0 and gi == 0),
                                    stop=(soff + g == T and gi == g - 1),
                                )

                    # ---------------- node phase for window w
                    z = nodep.tile([P, D], FP, tag="z")
                    if T > 0:
                        denc = nodep.tile([P, D], FP, tag="denc")
                        numc = nodep.tile([P, D], FP, tag="numc")
                        nc.vector.tensor_scalar(
                            out=denc[:], in0=acc_ps[:, 0:D],
                            scalar1=corr_sb[:, (l * 2) * W + w : (l * 2) * W + w + 1],
                            scalar2=1e-6, op0=ALU.subtract, op1=ALU.max,
                        )
                        nc.vector.tensor_scalar(
                            out=numc[:], in0=acc_ps[:, D : 2 * D],
                            scalar1=corr_sb[:, (l * 2 + 1) * W + w : (l * 2 + 1) * W + w + 1],
                            scalar2=None, op0=ALU.subtract,
                        )
                        rec = nodep.tile([P, D], FP, tag="rec")
                        nc.vector.reciprocal(rec[:], denc[:])
                        nc.vector.tensor_tensor(out=z[:], in0=numc[:], in1=rec[:], op=ALU.mult)
                        nc.vector.tensor_tensor(out=z[:], in0=z[:], in1=hin[:, wsl], op=ALU.add)
                    else:
                        nc.vector.tensor_copy(z[:], hin[:, wsl])

                    zT_ps = ps_node.tile([P, D], FP, tag="tp")
                    nc.tensor.transpose(zT_ps[:], z[:], ident[:])
                    zT = nodep.tile([P, D], FP, tag="zT")
                    nc.vector.tensor_copy(zT[:], zT_ps[:])
                    y1_ps = ps_node.tile([P, 2 * D], FP, tag="y1")
                    nc.tensor.matmul(y1_ps[:, 0:D], lhsT=w1a[:], rhs=zT[:], start=True, stop=True)
                    nc.tensor.matmul(y1_ps[:, D : 2 * D], lhsT=w1b[:], rhs=zT[:], start=True, stop=True)
                    r1 = nodep.tile([P, 2 * D], FP, tag="r1")
                    nc.scalar.activation(r1[:, 0:D], y1_ps[:, 0:D], AF.Relu, bias=b1a[:, 0:1])
                    nc.scalar.activation(r1[:, D : 2 * D], y1_ps[:, D : 2 * D], AF.Relu, bias=b1b[:, 0:1])
                    y2_ps = ps_node.tile([P, D], FP, tag="y2")
                    nc.tensor.matmul(y2_ps[:], lhsT=w2a[:], rhs=r1[:, 0:D], start=True, stop=False)
                    nc.tensor.matmul(y2_ps[:], lhsT=w2b[:], rhs=r1[:, D : 2 * D], start=False, stop=True)
                    y2b = nodep.tile([P, D], FP, tag="y2b")
                    nc.scalar.activation(y2b[:], y2_ps[:], AF.Identity, bias=b2c[:, 0:1])
                    hn_ps = ps_node.tile([P, D], FP, tag="tp")
                    nc.tensor.transpose(hn_ps[:], y2b[:], ident[:])
                    if l == 0:
                        nc.vector.tensor_copy(h_sb[:, wsl], hn_ps[:])
                    else:
                        nc.vector.tensor_tensor(
                            out=h_sb[:, wsl], in0=h_sb[:, wsl], in1=hn_ps[:], op=ALU.add
                        )

                    # LayerNorm(h_w) -> relu -> next-layer input / final head
                    hw = h_sb[:, wsl]
                    su = nodep.tile([P, 1], FP, tag="su")
                    nc.vector.reduce_sum(out=su[:], in_=hw, axis=AX.X)
                    mu = nodep.tile([P, 1], FP, tag="mu")
                    nc.scalar.mul(mu[:], su[:], 1.0 / D)
                    xc = nodep.tile([P, D], FP, tag="xc")
                    nc.vector.tensor_scalar(
                        out=xc[:], in0=hw, scalar1=mu[:, 0:1], scalar2=None,
                        op0=ALU.subtract,
                    )
                    sq = nodep.tile([P, D], FP, tag="sq")
                    ss = nodep.tile([P, 1], FP, tag="ss")
                    nc.scalar.activation(sq[:], xc[:], AF.Square, accum_out=ss[:])
                    sd = nodep.tile([P, 1], FP, tag="sd")
                    nc.scalar.activation(sd[:], ss[:], AF.Sqrt, scale=1.0 / D, bias=lneps_c[:, 0:1])
                    inv = nodep.tile([P, 1], FP, tag="inv")
                    nc.vector.reciprocal(inv[:], sd[:])

                    last = l == L - 1
                    if ln_affine[l]:
                        hnorm = nodep.tile([P, D], FP, tag="hnorm")
                        nc.vector.tensor_scalar(
                            out=hnorm[:], in0=xc[:], scalar1=inv[:, 0:1],
                            scalar2=None, op0=ALU.mult,
                        )
                        nc.vector.tensor_tensor(
                            out=hnorm[:], in0=hnorm[:],
                            in1=lnS_sb[:, l * D : (l + 1) * D], op=ALU.mult,
                        )
                        nc.vector.tensor_tensor(
                            out=hnorm[:], in0=hnorm[:],
                            in1=lnB_sb[:, l * D : (l + 1) * D], op=ALU.add,
                        )
                        if last:
                            hnf = nodep.tile([P, D], FP, tag="hnf")
                            nc.scalar.activation(hnf[:], hnorm[:], AF.Relu)
                        else:
                            nc.scalar.activation(hin[:, wsl], hnorm[:], AF.Relu)
                    else:
                        if last:
                            hnf = nodep.tile([P, D], FP, tag="hnf")
                            dest_ap = hnf[:]
                        else:
                            hnf = None
                            dest_ap = hin[:, wsl]
                        nc.vector.tensor_scalar(
                            out=dest_ap, in0=xc[:],
                            scalar1=inv[:, 0:1], scalar2=0.0,
                            op0=ALU.mult, op1=ALU.max,
                        )
                    if last:
                        yw = nodep.tile([P, D], FP, tag="yw")
                        nc.vector.tensor_tensor(
                            out=yw[:], in0=hnf[:], in1=woutT_sb[:, :], op=ALU.mult,
                        )
                        nc.vector.reduce_sum(out=y_sb[:, w : w + 1], in_=yw[:], axis=AX.X)
                    else:
                        nc.vector.tensor_copy(hin16[:, wsl], hin[:, wsl])
                    swin += T

                if l < L - 1:
                    nc.sync.dma_start(
                        out=hnloc[l][:, :].rearrange("(w p) d -> p w d", p=P),
                        in_=hin16[:].rearrange("p (w d) -> p w d", d=D),
                    )
                    nc.gpsimd.collective_compute(
                        "AllGather",
                        ALU.bypass,
                        replica_groups=[list(range(NCORES))],
                        ins=[hnloc[l][:, :]],
                        outs=[tabAG[l][1 : NG + 1, :]],
                    )

            # bout + writeout
            nc.vector.tensor_scalar(
                out=y_sb[:], in0=y_sb[:], scalar1=float(bout), scalar2=None,
                op0=ALU.add,
            )
            nc.sync.dma_start(
                out=y_d[:, :].rearrange("(w p) o -> p w o", p=P),
                in_=y_sb[:].rearrange("p (w o) -> p w o", o=1),
            )

    nc.compile()
    return nc


# ------------------------------------------------------------------- inputs

def _build_in_maps(inputs, lay):
    x = np.ascontiguousarray(np.asarray(inputs["x"], np.float32))
    ea = np.ascontiguousarray(np.asarray(inputs["edge_attr"], np.float32))
    W1 = np.ascontiguousarray(np.asarray(inputs["W1"], np.float32))
    b1 = np.asarray(inputs["b1"], np.float32).reshape(L, 2 * D, 1)
    W2 = np.ascontiguousarray(np.asarray(inputs["W2"], np.float32))
    b2 = np.asarray(inputs["b2"], np.float32).reshape(L, D, 1)
    beta = np.asarray(inputs["beta"], np.float32)
    ln_scale = np.asarray(inputs["ln_scale"], np.float32)
    ln_bias = np.asarray(inputs["ln_bias"], np.float32)
    lnf_scale = np.asarray(inputs["lnf_scale"], np.float32)
    lnf_bias = np.asarray(inputs["lnf_bias"], np.float32)
    Wout = np.asarray(inputs["Wout"], np.float32)

    n_pad, W, S = lay["n_pad"], lay["W"], lay["S"]
    NG = NCORES * n_pad
    NTAB = lay["NTAB"]
    node_at_pos, valid = lay["node_at_pos"], lay["valid"]

    xtab32 = np.zeros((NTAB, D), np.float32)
    xtab32[1 : NG + 1][valid] = x[node_at_pos[valid]]
    xtab = xtab32.astype(np.float16)

    # LN rows used: before conv l=1 -> ln[1]; l=2 -> ln[2]; final -> lnf.
    lnS = np.zeros((L, D), np.float32)
    lnB = np.zeros((L, D), np.float32)
    for l in range(L - 1):
        lnS[l] = ln_scale[l + 1]
        lnB[l] = ln_bias[l + 1]
    lnS[L - 1] = lnf_scale
    lnB[L - 1] = lnf_bias
    ln_affine = [
        not (np.all(lnS[l] == 1.0) and np.all(lnB[l] == 0.0)) for l in range(L)
    ]
    # replicate per-feature rows across all 128 partitions for DVE tensor_tensor
    lnS_rep = np.ascontiguousarray(np.tile(lnS.reshape(1, L * D), (P, 1)))
    lnB_rep = np.ascontiguousarray(np.tile(lnB.reshape(1, L * D), (P, 1)))
    wout_rep = np.ascontiguousarray(np.tile(Wout.reshape(1, D), (P, 1)))

    c_l = np.exp(beta * np.float32(EPS)).astype(np.float32)  # [L]

    in_maps = []
    for c in range(NCORES):
        rows = lay["ea_rows"][c]
        ea_c = ea[np.clip(rows, 0, None)].astype(np.float16)
        ea_c[rows < 0] = 0.0
        corr = np.zeros((P, L * 2 * W), np.float32)
        for l in range(L):
            corr[:, (l * 2) * W : (l * 2 + 1) * W] = lay["npad"][c] * c_l[l]
            corr[:, (l * 2 + 1) * W : (l * 2 + 2) * W] = (
                lay["npad"][c] * c_l[l] * np.float32(EPS)
            )
        hin0 = (
            xtab32[1 + c * n_pad : 1 + (c + 1) * n_pad]
            .reshape(W, P, D)
            .transpose(1, 0, 2)
            .reshape(P, W * D)
        )
        in_maps.append(
            {
                "xtab": xtab,
                "ea": np.ascontiguousarray(ea_c),
                "gidx": np.ascontiguousarray(lay["gidx"][c]),
                "corr": corr,
                "hin0": np.ascontiguousarray(hin0),
                "W1": W1,
                "W2": W2,
                "b1": np.ascontiguousarray(b1),
                "b2": np.ascontiguousarray(b2),
                "lnS": lnS_rep,
                "lnB": lnB_rep,
                "woutT": wout_rep,
            }
        )
    meta = dict(
        betas=[float(b) for b in beta],
        bout=float(np.asarray(inputs["bout"]).reshape(-1)[0]),
        ln_affine=ln_affine,
    )
    return in_maps, meta


_CACHE = {}


def _get_program(inputs):
    edge_index = np.asarray(inputs["edge_index"])
    key = hash(
        (
            edge_index.tobytes(),
            np.asarray(inputs["beta"], np.float32).tobytes(),
            np.asarray(inputs["bout"], np.float32).tobytes(),
            np.asarray(inputs["ln_scale"], np.float32).tobytes(),
            np.asarray(inputs["ln_bias"], np.float32).tobytes(),
            np.asarray(inputs["lnf_scale"], np.float32).tobytes(),
            np.asarray(inputs["lnf_bias"], np.float32).tobytes(),
        )
    )
    if key not in _CACHE:
        n_nodes = np.asarray(inputs["x"]).shape[0]
        lay = _host_layout(edge_index, n_nodes)
        in_maps, meta = _build_in_maps(inputs, lay)
        nc = _build_program(lay, meta["betas"], meta["bout"], meta["ln_affine"])
        _CACHE[key] = (nc, lay)
        return nc, lay, in_maps
    nc, lay = _CACHE[key]
    in_maps, _ = _build_in_maps(inputs, lay)
    return nc, lay, in_maps


def kernel(**inputs) -> np.ndarray:
    nc, lay, in_maps = _get_program(inputs)
    res = run_bass_kernel_spmd(nc, in_maps, list(range(NCORES)))
    results = res.results
    n_pad = lay["n_pad"]
    ys = np.concatenate([results[c]["y"] for c in range(NCORES)], axis=0)
    out = np.zeros((lay["n_nodes"], 1), np.float32)
    valid = lay["valid"]
    out[lay["node_at_pos"][valid]] = ys[valid]
    return out

